# revision 64
# baseline (speedup 1.0000x reference)
"""DGCNN forward kernel for Trainium2, data-parallel over 8 NeuronCores.

Structure of the problem (shapes hardcoded from the task spec):
  x          [1_000_000, 64]  fp32   node features, 10_000 graphs x 100 nodes
  batch      arange(N)//100          (graphs are contiguous 100-node blocks)
  4-layer MLP 64->64->64->64->34 with ReLU
  mean-pool over the FIRST 30 nodes of each graph  -> [10_000, 34]
  conv1d(1->16,k=5) + ReLU -> maxpool(2) -> conv1d(16->32,k=5) + ReLU
  flatten -> linear(352->2)

v5 design (cost-model driven; engines are in-order so emission order and
PSUM ring sizing dominate the schedule):
  * bf16 everywhere; two node halves packed into 128 SBUF partitions with
    block-diagonal MLP weights; tiles [128, 1920] = 4 chunks x 480 cols;
    wave depth 1 (L_i of tile t runs one wave after L_{i-1}).
  * relu(+bias) runs as 960-col pair ops split between ACT and DVE (the
    only engines that may read PSUM) by an exact-cost greedy planner with
    a DVE bias plus a small searched force-map.
  * pooling: mode 'v' tiles do the 30->15 halve on DVE (2x bf16 mode) and
    the rest of the chain on the Pool engine; mode 'F' tiles fuse the L4
    relu with the halve via scalar_tensor_tensor (valid when b4 == 0,
    which setup_inputs guarantees; a 'noF' variant covers b4 != 0); 'd'
    tiles (the 49-graph drain tile) use a short all-DVE path.
  * the head runs one 64-graph slot per tile, pipelined over three waves
    (conv1+relu/maxpool, conv2+relu, fc+copy+DMA); the three conv bands
    of both halves land in single PSUM banks so one batched op covers
    them; band constants are zero-padded to 128 outputs so batched ops
    never read stale PSUM.
  * a matmul whose operands sit at base partition 64 may only write PSUM
    below byte offset 512 (discovered on hw), so half B's pooled rows are
    DMA-lane-shifted to partitions 0:34 and the whole head runs base-0.
  * per-slot output DMAs overlap the MLP waves; y is slot-major
    [A 64 | B 64] per tile and reassembled on the host.
"""

import json

import numpy as np

# ---------------------------------------------------------------- constants
N = 1_000_000
G = 10_000
NODES_PER_G = 100
K = 30
F = 64
NCORE = 8
G_CORE = G // NCORE          # 1250 graphs per core
G_HALF = G_CORE // 2         # 625 graphs per packed half
G_HALF_PAD = 640             # padded to 10 tiles of 64 graphs
TILE_G = 64                  # graphs per half per tile
N_TILES = G_HALF_PAD // TILE_G
N_CHUNKS = 4                 # 480-col matmul chunks per tile
DATA_COLS = 480              # 16 graphs * 30 nodes
TILE_COLS = N_CHUNKS * DATA_COLS    # 1920
PS_STRIDE = 512              # psum chunk stride (bank = 512 fp32)
LAST_G = G_HALF - (N_TILES - 1) * TILE_G   # 49 real graphs in tile 9
Y_COLS = 2 * G_HALF_PAD      # y dram: per tile i, [A 64 | B 64] at col 128*i

# const tensor column offsets (bf16, [128, NCOL]); conv1/conv2 band lhsT
# blocks are zero-padded to 128 outputs so batched relu ops read no stale
# PSUM rows.
OFF_W1, OFF_W2, OFF_W3, OFF_W4 = 0, 128, 256, 384   # W4: 98 cols
OFF_CE1, OFF_CO1 = 482, 610       # conv1 even/odd t 0..15
OFF_CEM, OFF_COM = 738, 866       # conv1 even/odd t 8..23
OFF_CE2, OFF_CO2 = 994, 1122      # conv1 even/odd t 16..29 (padded)
OFF_C20, OFF_C21, OFF_C22 = 1250, 1378, 1506   # conv2 bands (padded)
OFF_WO0, OFF_WO1, OFF_WO2 = 1634, 1636, 1638
OFF_ID = 1640
NCOL = 1674
# fp32 bias tensor columns ([128, 8])
FB_B1, FB_B2, FB_B3, FB_B4, FB_C1B, FB_C2B = 0, 1, 2, 3, 4, 5


# ------------------------------------------------- walrus sync-wait workaround
def _split_sync_waits(bir: dict) -> dict:
    """The walrus build in this container accepts at most ONE sync-wait per
    instruction.  Hoist extra waits onto same-engine EventSemaphore carriers
    (the exact shape wait_ge() emits) inserted right before the instruction;
    engines dispatch in order so semantics are unchanged."""
    for fn in bir.get("functions", []):
        for bb in fn.get("blocks", []):
            out = []
            for inst in bb.get("instructions", []):
                si = inst.get("sync_info") or {}
                ow = si.get("on_wait") or []
                if len(ow) > 1:
                    for k, w in enumerate(ow[:-1]):
                        out.append(
                            {
                                "debug": inst.get("debug"),
                                "engine": inst["engine"],
                                "ins": [],
                                "name": f"{inst['name']}_hw{k}",
                                "opcode": "EventSemaphore",
                                "outs": [],
                                "sync_info": {"on_update": [], "on_wait": [w]},
                            }
                        )
                    si = dict(si)
                    si["on_wait"] = [ow[-1]]
                    inst = dict(inst)
                    inst["sync_info"] = si
                out.append(inst)
            bb["instructions"] = out
    return bir


_patch_installed = False


def _install_bir_patch():
    global _patch_installed
    if _patch_installed:
        return
    import concourse.bass as bass

    orig = bass.Bass.to_json_bytes
    if getattr(bass.Bass, "_ant_sync_wait_patch", False):
        _patch_installed = True
        return

    def patched(self) -> bytes:
        return json.dumps(_split_sync_waits(json.loads(orig(self)))).encode()

    bass.Bass.to_json_bytes = patched
    bass.Bass._ant_sync_wait_patch = True
    _patch_installed = True


# tuning knobs (read by _build_nc; override before first _get_nc call)
TUNE = {
    # per-tile pooling: 'F' = L4 relu fused with halve1 (needs b4 == 0; one
    # relu-odd op + one stt op, then a 4-stage Pool chain), 'P' = plain L4
    # relu + 5-stage Pool chain, 'h' = Pool halves + DVE reduce8, 'd' = DVE
    # halve + reduce15
    "pool_modes": "vvvvvvvFFd",
    "ps_bufs": 3,
    "hs_bufs": 2,
    "x0_split": 2,
    "wave_depth": 1,
    "hp_bufs": 12,
    "x_queues": "gs" + "s" * 8,   # per-tile DMA queue: 'g' routes tile 0
                                  # through gpsimd SWDGE (skips the HWDGE+DGE
                                  # startup latency); 's'/'a' are the SP/ACT
                                  # HWDGE queues
    "stage_off": None,        # head stage1 wave offset (default 3*D+3)
    "singles": False,         # per-chunk psum tiles + relus (ps_bufs=6)
    "dve_bias": 300.0,          # planner penalty for placing flexible ops on DVE
    "hsb_bufs": 4,
    "php_bufs": 2,
    "y_last_g": False,
    "ls_last": False,         # PE lane shift for the last tile
    "head_after": True,       # emit head stages after the wave's MLP ops
    # (kind, key) -> 'a'/'d' planner overrides (from a local search on the
    # cost-model timeline)
    "force": {("se", 0): "a", ("se", 2): "a", ("relu", 1, 3, 1): "a",
              ("relu", 6, 1, 1): "a", ("relu", 2, 0, 1): "a",
              ("relu", 1, 1, 0): "d", ("se", 6): "d",
              # last two head slots on opposite engines so their drain
              # chains overlap
              ("se", 9): "d", ("r2", 9): "d", ("cpy", 9): "d",
              ("se", 8): "a", ("r2", 8): "a", ("cpy", 8): "a"},
}

# exact cost-model numbers (ns) used by the planner
def _relu_cost(eng, cols):
    if eng == "a":
        return cols * 0.8333 + 185.0
    return cols * 1.0417 + 125.0


def _plan_engines():
    """Greedy wave-ordered ACT/DVE assignment with exact cost-model costs.

    Returns dict job-key -> 'a'/'d'.  Job keys:
      ('relu', i, li, gi)  MLP relu pair ops
      ('se', slot, half), ('r2', slot, half), ('cpy', slot)
    stt is DVE-only, pooling is fixed by pool_modes.
    """
    D = TUNE["wave_depth"]
    force = TUNE.get("force", {})
    load = {"a": 0.0, "d": 0.0}
    assign = {}

    bias = {"a": 0.0, "d": TUNE.get("dve_bias", 0.0)}

    def pick(key, costs):
        if key in force:
            e = force[key]
        else:
            e = min(costs, key=lambda k: load[k] + costs[k] + bias[k])
        load[e] += costs[e]
        assign[key] = e
        return e

    SOFF = TUNE.get("stage_off") or (3 * D + 3)
    for w in range(N_TILES + SOFF + 3):
        # head stage ops (one slot per tile, both halves in one op)
        s1 = w - SOFF
        if 0 <= s1 < N_TILES:
            pick(("se", s1), {"a": 384 * 0.8333 + 185, "d": 384 * 1.0417 + 125})
            load["d"] += 384 * 1.0417 + 125                  # stt, DVE-only
        s2 = s1 - 1
        if 0 <= s2 < N_TILES:
            pick(("r2", s2), {"a": 384 * 0.8333 + 185, "d": 384 * 1.0417 + 125})
        s3 = s1 - 2
        if 0 <= s3 < N_TILES:
            pick(("cpy", s3), {"a": 128 * 0.8333 + 185, "d": 128 * 1.0417 + 125})
        for li in range(4):
            i = w - li * D
            if 0 <= i < N_TILES:
                if li == 3 and TUNE["pool_modes"][i] == "F":
                    for gi in range(2):
                        pick(("l4o", i, gi),
                             {"a": 480 * 0.8333 + 185, "d": 480 * 1.0417 + 125})
                        load["d"] += 2 * (240 * 1.0417 + 125)   # stt x2, DVE
                    continue
                ncols = TILE_COLS if i < N_TILES - 1 else LAST_G * K
                for gi in range(2):
                    cols = min(2 * DATA_COLS, max(0, ncols - gi * 2 * DATA_COLS))
                    if cols <= 0:
                        continue
                    pick(("relu", i, li, gi),
                         {"a": _relu_cost("a", cols), "d": _relu_cost("d", cols)})
        ip = w - 3 * D - 1
        if 0 <= ip < N_TILES:
            ng = TILE_G if ip < N_TILES - 1 else LAST_G
            mode = TUNE["pool_modes"][ip]
            if mode == "d":
                load["d"] += ng * 15 * 0.52 + 125 + ng * 15 * 1.0417 + 185
            elif mode == "h":
                load["d"] += ng * 8 * 1.0417 + 185
            elif mode == "v":
                load["d"] += ng * 15 * 0.52 + 125
    return assign


# ------------------------------------------------------------- device program
_NC_CACHE = {}


def _build_nc():
    """Build the per-core Bass program (identical on all 8 cores)."""
    _install_bir_patch()
    import concourse.bass as bass
    import concourse.tile as tile
    from concourse import mybir

    f32 = mybir.dt.float32
    bf16 = mybir.dt.bfloat16
    Relu = mybir.ActivationFunctionType.Relu
    ADD = mybir.AluOpType.add
    MAX = mybir.AluOpType.max
    AX = mybir.AxisListType.X

    nc = bass.Bass()
    xt = nc.dram_tensor("xt", [N_TILES, 128, TILE_COLS], bf16, kind="ExternalInput")
    cst = nc.dram_tensor("cst", [128, NCOL], bf16, kind="ExternalInput")
    cstf = nc.dram_tensor("cstf", [128, 8], f32, kind="ExternalInput")
    y = nc.dram_tensor("y", [2, Y_COLS], f32, kind="ExternalOutput")

    plan = _plan_engines()
    D = TUNE["wave_depth"]

    with tile.TileContext(nc) as tc:
        with (
            tc.tile_pool(name="persist", bufs=1) as persist,
            tc.tile_pool(name="xp", bufs=3) as xp,
            tc.tile_pool(name="hp", bufs=TUNE["hp_bufs"]) as hp,
            tc.tile_pool(name="hsb", bufs=TUNE.get("hsb_bufs", 4)) as hsb,
        ):
            cstt = persist.tile([128, NCOL], bf16)
            cstf_t = persist.tile([128, 8], f32)
            # DMA order on the SP queue: MLP weights -> x tile 0 -> biases ->
            # head constants -> x tiles 1..9 (the serialized HWDGE FIFO makes
            # position = priority).
            nc.sync.dma_start(out=cstt[:, 0:OFF_W2], in_=cst[:, 0:OFF_W2])
            nc.sync.dma_start(out=cstt[:, OFF_W2:OFF_CE1], in_=cst[:, OFF_W2:OFF_CE1])
            xq = TUNE.get("x_queues") or "s" * N_TILES
            q_eng = {"s": nc.sync, "a": nc.scalar, "d": nc.vector, "g": nc.gpsimd}
            xt_sb = []
            for i in range(N_TILES):
                xt_sb.append(xp.tile([128, TILE_COLS], bf16, name="xt_i"))
            nsp0 = TUNE.get("x0_split", 4)
            step0 = TILE_COLS // nsp0
            for cq in range(nsp0):
                q_eng[xq[0]].dma_start(
                    out=xt_sb[0][:, cq * step0 : (cq + 1) * step0],
                    in_=xt[0][:, cq * step0 : (cq + 1) * step0],
                )
            nc.sync.dma_start(out=cstf_t[:], in_=cstf[:, :])
            for i in range(1, N_TILES):
                q_eng[xq[i]].dma_start(out=xt_sb[i][:], in_=xt[i])
            # head constants are not needed until the first head slot: issue
            # them after the whole x stream.
            nc.sync.dma_start(out=cstt[:, OFF_CE1:NCOL], in_=cst[:, OFF_CE1:NCOL])

            pooledP = persist.tile([98, G_HALF_PAD], bf16)  # A rows 0:34, B 64:98
            nc.vector.memset(pooledP[:, G_HALF:G_HALF_PAD], 0.0)
            # half B pooled rows copied down to partitions 0:34 per tile (a
            # base-64 matmul may only write PSUM below byte offset 512, so
            # the head runs all-base-0 instead)
            pooledB = persist.tile([34, G_HALF_PAD], bf16)
            ysb = persist.tile([2, Y_COLS], f32)

            layer_cfg = [
                (OFF_W1, 128, FB_B1, 128),
                (OFF_W2, 128, FB_B2, 128),
                (OFF_W3, 128, FB_B3, 128),
                (OFF_W4, 98, FB_B4, 98),
            ]

            def relu_op(eng, out_v, in_v, b_ap):
                if eng == "a":
                    nc.scalar.activation(out_v, in_v, Relu, bias=b_ap)
                else:
                    nc.vector.tensor_scalar(out_v, in_v, b_ap, 0.0, ADD, MAX)

            h_tiles = {}

            def mlp_layer(i, li, mps):
                """Emit matmuls + relus for (tile i, layer li)."""
                ncols = TILE_COLS if i < N_TILES - 1 else LAST_G * K
                cur = xt_sb[i] if li == 0 else h_tiles[(i, li - 1)]
                woff, wm, boff, outp = layer_cfg[li]
                w_ap = cstt[:, woff : woff + wm]
                b_ap = cstf_t[0:outp, boff : boff + 1]
                fuse = li == 3 and TUNE["pool_modes"][i] == "F"
                if fuse:
                    # L4 relu fused with the 30->15 halve (b4 == 0): per
                    # pair, relu the odd half-window then stt-add the even
                    # half on top, writing the halved tile directly.
                    h = hp.tile([98, TILE_G * 15], bf16, name="hh",
                                tag="hhf", bufs=4)
                else:
                    h = hp.tile([128, TILE_COLS], bf16, name="h")
                h_tiles[(i, li)] = h
                singles = TUNE.get("singles", False)
                if fuse and singles:
                    # one psum bank per chunk; relu-odd + stt per chunk (3D)
                    for c in range(4):
                        ps = mps.tile([128, PS_STRIDE], mybir.dt.float32,
                                      name="ps", tag="ps2",
                                      bufs=TUNE["ps_bufs"])
                        nc.tensor.matmul(
                            ps[0:outp, 0:DATA_COLS], w_ap,
                            cur[:, c * DATA_COLS : c * DATA_COLS + DATA_COLS],
                            start=True, stop=True)
                        psvc = ps[0:98, 0:DATA_COLS].rearrange(
                            "p (g k) -> p g k", k=K)
                        h4o = php.tile([98, 240], bf16, name="h4o",
                                       tag="h4o", bufs=5)
                        h4ov = h4o.rearrange("p (g k) -> p g k", k=15)
                        eng = plan[("l4o", i, c // 2)]
                        if eng == "a":
                            nc.scalar.activation(h4ov, psvc[:, :, 15:30], Relu)
                        else:
                            nc.vector.tensor_scalar(
                                h4ov, psvc[:, :, 15:30], 0.0, 0.0, MAX, ADD)
                        hvc = h[:, c * 240 : c * 240 + 240].rearrange(
                            "p (g k) -> p g k", k=15)
                        with nc.allow_low_precision(reason="halved sums bf16"):
                            nc.vector.scalar_tensor_tensor(
                                hvc, psvc[:, :, 0:15], 0.0, h4ov, MAX, ADD)
                    return
                if fuse:
                    for gi, c0 in enumerate(range(0, 4, 2)):
                        ps = mps.tile(
                            [128, 2 * PS_STRIDE], mybir.dt.float32, name="ps",
                            tag="ps2", bufs=TUNE["ps_bufs"],
                        )
                        for g in range(2):
                            c = c0 + g
                            nc.tensor.matmul(
                                ps[0:outp, g * PS_STRIDE : g * PS_STRIDE + DATA_COLS],
                                w_ap,
                                cur[:, c * DATA_COLS : c * DATA_COLS + DATA_COLS],
                                start=True,
                                stop=True,
                            )
                        psv = ps[0:98, :].rearrange(
                            "p (c s) -> p c s", c=2)[:, :, 0:DATA_COLS].rearrange(
                            "p c (g k) -> p c g k", k=K)
                        h4o = php.tile([98, 2 * 16 * 15], bf16, name="h4o",
                                       tag="h4o", bufs=3)
                        h4ov = h4o.rearrange("p (c g k) -> p c g k", c=2, g=16)
                        eng = plan[("l4o", i, gi)]
                        if eng == "a":
                            nc.scalar.activation(h4ov, psv[:, :, :, 15:30], Relu)
                        else:
                            nc.vector.tensor_scalar(
                                h4ov, psv[:, :, :, 15:30], 0.0, 0.0, MAX, ADD)
                        # stt is limited to 3D inputs: one op per 480-chunk
                        with nc.allow_low_precision(reason="halved sums bf16"):
                            for cc in range(2):
                                pse = ps[0:98, cc * PS_STRIDE :
                                         cc * PS_STRIDE + DATA_COLS].rearrange(
                                    "p (g k) -> p g k", k=K)[:, :, 0:15]
                                h4oc = h4o[:, cc * 240 : cc * 240 + 240].rearrange(
                                    "p (g k) -> p g k", k=15)
                                hvc = h[:, gi * 480 + cc * 240 :
                                        gi * 480 + cc * 240 + 240].rearrange(
                                    "p (g k) -> p g k", k=15)
                                nc.vector.scalar_tensor_tensor(
                                    hvc, pse, 0.0, h4oc, MAX, ADD)
                    return
                if singles:
                    for c in range(4):
                        dc = min(DATA_COLS, ncols - c * DATA_COLS)
                        if dc <= 0:
                            continue
                        ps = mps.tile([128, PS_STRIDE], mybir.dt.float32,
                                      name="ps", tag="ps2",
                                      bufs=TUNE["ps_bufs"])
                        nc.tensor.matmul(
                            ps[0:outp, 0:dc], w_ap,
                            cur[:, c * DATA_COLS : c * DATA_COLS + dc],
                            start=True, stop=True)
                        relu_op(plan[("relu", i, li, c // 2)],
                                h[0:outp, c * DATA_COLS : c * DATA_COLS + dc],
                                ps[0:outp, 0:dc], b_ap)
                    return
                for gi, c0 in enumerate(range(0, 4, 2)):
                    lo = c0 * DATA_COLS
                    if lo >= ncols:
                        continue
                    ps = mps.tile(
                        [128, 2 * PS_STRIDE], mybir.dt.float32, name="ps",
                        tag="ps2", bufs=TUNE["ps_bufs"],
                    )
                    dc_full = True
                    for g in range(2):
                        c = c0 + g
                        dc = min(DATA_COLS, ncols - c * DATA_COLS)
                        if dc <= 0:
                            dc_full = False
                            continue
                        if dc < DATA_COLS:
                            dc_full = False
                        nc.tensor.matmul(
                            ps[0:outp, g * PS_STRIDE : g * PS_STRIDE + dc],
                            w_ap,
                            cur[:, c * DATA_COLS : c * DATA_COLS + dc],
                            start=True,
                            stop=True,
                        )
                    eng = plan[("relu", i, li, gi)]
                    if dc_full:
                        ps_v = ps[0:outp, :].rearrange(
                            "p (c s) -> p c s", c=2
                        )[:, :, 0:DATA_COLS]
                        h_v = h[
                            0:outp, lo : lo + 2 * DATA_COLS
                        ].rearrange("p (c s) -> p c s", c=2)
                        relu_op(eng, h_v, ps_v, b_ap)
                    else:
                        for g in range(2):
                            c = c0 + g
                            dc = min(DATA_COLS, ncols - c * DATA_COLS)
                            if dc <= 0:
                                continue
                            relu_op(
                                eng,
                                h[0:outp, c * DATA_COLS : c * DATA_COLS + dc],
                                ps[0:outp, g * PS_STRIDE : g * PS_STRIDE + dc],
                                b_ap,
                            )

            def pool_tile(i, php):
                """Mean over 30 nodes per graph -> pooledP[:, i*64 : ...].

                Mode 'P': 5-stage halving chain on the Pool engine (zero
                ACT/DVE cost).  Mode 'd': halve + 15-wide reduce on DVE.
                The 1/30 is folded into the conv1 constants.
                """
                ng = TILE_G if i < N_TILES - 1 else LAST_G
                cur = h_tiles.pop((i, 3))
                pout = pooledP[:, i * TILE_G : i * TILE_G + ng]
                mode = TUNE["pool_modes"][i]
                with nc.allow_low_precision(reason="pooled sums fit bf16"):
                    if mode == "F":
                        # cur is already the halved [98, 64*15] tile
                        hhv = cur[0:98, :].rearrange("p (g k) -> p g k", k=15)
                    elif mode == "v":
                        # halve1 on DVE (2x bf16 mode), rest of chain on Pool
                        h4v = cur[0:98, 0 : ng * K].rearrange(
                            "p (g k) -> p g k", k=K)
                        hh = php.tile([98, TILE_G * 15], bf16, name="hh")
                        hhv = hh[:, 0 : ng * 15].rearrange(
                            "p (g k) -> p g k", k=15)
                        nc.vector.tensor_tensor(
                            hhv, h4v[:, :, 0:15], h4v[:, :, 15:30], op=ADD)
                    else:
                        h4v = cur[0:98, 0 : ng * K].rearrange(
                            "p (g k) -> p g k", k=K)
                        if mode == "d":
                            hh = php.tile([98, TILE_G * 15], bf16, name="hh")
                            hhv = hh[:, 0 : ng * 15].rearrange(
                                "p (g k) -> p g k", k=15)
                            nc.vector.tensor_tensor(
                                hhv, h4v[:, :, 0:15], h4v[:, :, 15:30], op=ADD)
                            nc.vector.tensor_reduce(pout, hhv, axis=AX, op=ADD)
                            return
                        hh = php.tile([98, TILE_G * 15], bf16, name="hh")
                        hhv = hh[:, 0 : ng * 15].rearrange(
                            "p (g k) -> p g k", k=15)
                        nc.gpsimd.tensor_tensor(
                            hhv, h4v[:, :, 0:15], h4v[:, :, 15:30], op=ADD)
                    h8 = php.tile([98, TILE_G * 8], bf16, name="h8")
                    h8v = h8[:, 0 : ng * 8].rearrange("p (g k) -> p g k", k=8)
                    nc.gpsimd.tensor_tensor(
                        h8v[:, :, 0:7], hhv[:, :, 0:7], hhv[:, :, 7:14], op=ADD)
                    nc.gpsimd.tensor_copy(h8v[:, :, 7:8], hhv[:, :, 14:15])
                    if mode == "h":
                        nc.vector.tensor_reduce(pout, h8v, axis=AX, op=ADD)
                        return
                    h4 = php.tile([98, TILE_G * 4], bf16, name="h4")
                    h4vv = h4[:, 0 : ng * 4].rearrange("p (g k) -> p g k", k=4)
                    nc.gpsimd.tensor_tensor(
                        h4vv, h8v[:, :, 0:4], h8v[:, :, 4:8], op=ADD)
                    h2 = php.tile([98, TILE_G * 2], bf16, name="h2")
                    h2v = h2[:, 0 : ng * 2].rearrange("p (g k) -> p g k", k=2)
                    nc.gpsimd.tensor_tensor(
                        h2v, h4vv[:, :, 0:2], h4vv[:, :, 2:4], op=ADD)
                    nc.gpsimd.tensor_tensor(
                        pout.rearrange("p (g k) -> p g k", k=1),
                        h2v[:, :, 0:1], h2v[:, :, 1:2], op=ADD)

            conv1_cfg = [(OFF_CE1, OFF_CO1), (OFF_CEM, OFF_COM), (OFF_CE2, OFF_CO2)]
            conv2_cfg = [(OFF_C20, 0, 128), (OFF_C21, 1, 128), (OFF_C22, 2, 112)]

            def hs_op(key, out_v, in_v, b_ap):
                if plan[key] == "a":
                    nc.scalar.activation(out_v, in_v, Relu, bias=b_ap)
                else:
                    nc.vector.tensor_scalar(out_v, in_v, b_ap, 0.0, ADD, MAX)

            hstate = {}
            HC = TILE_G  # 64 graphs per head slot (one slot per tile)

            def head_stage1(i, mps):
                """conv1 matmuls (both halves into one PSUM tile) + one
                relu(even) + one maxpool(odd) op covering both halves."""
                c0 = i * HC
                c1b = cstf_t[0:128, FB_C1B : FB_C1B + 1]
                # 1-bank tiles: even bands of half h at [h*256 + b*64]
                pe = mps.tile([128, PS_STRIDE], mybir.dt.float32,
                              name="pe", tag="hs", bufs=TUNE["hs_bufs"])
                po = mps.tile([128, PS_STRIDE], mybir.dt.float32,
                              name="po", tag="hs", bufs=TUNE["hs_bufs"])
                for half in range(2):
                    hoff = half * 256
                    rhs = (pooledP[0:34, c0 : c0 + HC] if half == 0
                           else pooledB[0:34, c0 : c0 + HC])
                    for bi, (offE, offO) in enumerate(conv1_cfg):
                        nc.tensor.matmul(
                            pe[0:128, hoff + bi * HC : hoff + bi * HC + HC],
                            cstt[0:34, offE : offE + 128], rhs,
                            start=True, stop=True)
                        nc.tensor.matmul(
                            po[0:128, hoff + bi * HC : hoff + bi * HC + HC],
                            cstt[0:34, offO : offO + 128], rhs,
                            start=True, stop=True)
                pev = pe.rearrange("p (h c) -> p h c", h=2)[:, :, 0:192]
                pov = po.rearrange("p (h c) -> p h c", h=2)[:, :, 0:192]
                se = hsb.tile([128, 2 * 192], bf16, name="se")
                sev = se.rearrange("p (h c) -> p h c", h=2)
                hs_op(("se", i), sev, pev, c1b)
                ms = hsb.tile([128, 2 * 192], bf16, name="ms")
                msv = ms.rearrange("p (h c) -> p h c", h=2)
                nc.vector.scalar_tensor_tensor(msv, pov, c1b, sev, ADD, MAX)
                hstate[i] = {"ms": ms}

            def head_stage2(i, mps):
                """conv2 matmuls + one relu op covering both halves."""
                c2b = cstf_t[0:128, FB_C2B : FB_C2B + 1]
                ms = hstate[i]["ms"]
                p2 = mps.tile([128, PS_STRIDE], mybir.dt.float32, name="p2",
                              tag="hs", bufs=TUNE["hs_bufs"])
                for half in range(2):
                    for ci, (off, src, kk) in enumerate(conv2_cfg):
                        nc.tensor.matmul(
                            p2[0:128, half * 256 + ci * HC :
                               half * 256 + ci * HC + HC],
                            cstt[0:kk, off : off + 128],
                            ms[0:kk, half * 192 + src * HC :
                               half * 192 + src * HC + HC],
                            start=True, stop=True)
                p2v = p2.rearrange("p (h c) -> p h c", h=2)[:, :, 0:192]
                rs = hsb.tile([128, 2 * 192], bf16, name="rs")
                rsv = rs.rearrange("p (h c) -> p h c", h=2)
                hs_op(("r2", i), rsv, p2v, c2b)
                hstate[i]["rs"] = rs

            def head_stage3(i, mps):
                """fc matmuls + one psum->sbuf copy + output DMA."""
                c0 = i * HC
                rs = hstate.pop(i)["rs"]
                py = mps.tile([2, PS_STRIDE], mybir.dt.float32, name="py",
                              tag="hs", bufs=TUNE["hs_bufs"])
                for half in range(2):
                    for gi, (off, m) in enumerate(
                        zip([OFF_WO0, OFF_WO1, OFF_WO2], [128, 128, 96])
                    ):
                        nc.tensor.matmul(
                            py[:, half * HC : half * HC + HC],
                            cstt[0:m, off : off + 2],
                            rs[0:m, half * 192 + gi * HC :
                               half * 192 + gi * HC + HC],
                            start=(gi == 0), stop=(gi == 2))
                yc = ysb[:, 2 * c0 : 2 * c0 + 2 * HC]
                if plan[("cpy", i)] == "a":
                    nc.scalar.add(yc, py[:, 0 : 2 * HC], add=0.0)
                else:
                    nc.vector.tensor_scalar(yc, py[:, 0 : 2 * HC], 0.0, 0.0,
                                            ADD, ADD)
                # the last slot's DMA is the drain tail: route it through
                # the software DGE on the (idle at drain) Pool queue, which
                # has a shorter fixed-latency chain than SP HWDGE
                yq = nc.gpsimd if (i == N_TILES - 1 and TUNE.get("y_last_g",
                                                                 True)) else nc.sync
                yq.dma_start(out=y[:, 2 * c0 : 2 * c0 + 2 * HC], in_=yc)

            with (
                tc.tile_pool(name="mps", bufs=TUNE["ps_bufs"], space="PSUM") as mps,
                tc.tile_pool(name="php", bufs=TUNE.get("php_bufs", 2)) as php,
            ):
                stages = [head_stage1, head_stage2, head_stage3]
                SOFF = TUNE.get("stage_off") or (3 * D + 3)
                for w in range(N_TILES + SOFF + 3):
                    # head stages first (their deps are a wave old, so they
                    # are ready and fill engine time while matmuls run);
                    # oldest stage first so ring slots free before reuse.
                    # stage1 of tile i runs one wave after pool(i).
                    def emit_head():
                        for s in (2, 1, 0):
                            sp = w - SOFF - s
                            if 0 <= sp < N_TILES:
                                stages[s](sp, mps)
                    if not TUNE.get("head_after"):
                        emit_head()
                    for li in range(4):
                        i = w - li * D
                        if 0 <= i < N_TILES:
                            mlp_layer(i, li, mps)
                    if TUNE.get("head_after"):
                        emit_head()
                    ip = w - 3 * D - 1
                    if 0 <= ip < N_TILES:
                        pool_tile(ip, php)
                        # lane-shift half B's pooled rows to partitions 0:34;
                        # the last tile uses a PE identity matmul (short
                        # latency) instead of an SBUF->SBUF DMA so the drain
                        # does not wait ~2.2us of DMA fixed overheads
                        if ip == N_TILES - 1 and TUNE.get("ls_last"):
                            pls = mps.tile([34, PS_STRIDE], mybir.dt.float32,
                                           name="pls", tag="hs",
                                           bufs=TUNE["hs_bufs"])
                            nc.tensor.matmul(
                                pls[0:34, 0:TILE_G],
                                cstt[64:98, OFF_ID : OFF_ID + 34],
                                pooledP[64:98, ip * TILE_G : (ip + 1) * TILE_G],
                                start=True, stop=True)
                            nc.scalar.add(
                                pooledB[0:34, ip * TILE_G : (ip + 1) * TILE_G],
                                pls[0:34, 0:TILE_G], add=0.0)
                        else:
                            nc.sync.dma_start(
                                out=pooledB[0:34, ip * TILE_G : (ip + 1) * TILE_G],
                                in_=pooledP[64:98, ip * TILE_G : (ip + 1) * TILE_G])
    return nc


def _get_nc(variant="F"):
    key = ("nc", variant)
    if key not in _NC_CACHE:
        if variant == "noF":
            saved = TUNE["pool_modes"]
            TUNE["pool_modes"] = saved.replace("F", "P")
            try:
                _NC_CACHE[key] = _build_nc()
            finally:
                TUNE["pool_modes"] = saved
        else:
            _NC_CACHE[key] = _build_nc()
    return _NC_CACHE[key]


# ------------------------------------------------------------------ host prep
def _prep_x(x):
    """[N, 64] fp32 -> per-core [N_TILES, 128, 1920] bf16 transposed tiles."""
    import ml_dtypes

    xs = np.ascontiguousarray(x.reshape(G, NODES_PER_G, F)[:, :K, :])
    xs = xs.astype(ml_dtypes.bfloat16)
    out = np.zeros((NCORE, N_TILES, 128, TILE_COLS), ml_dtypes.bfloat16)
    for c in range(NCORE):
        for half in range(2):
            gs = c * G_CORE + half * G_HALF
            segp = np.zeros((G_HALF_PAD, K, F), ml_dtypes.bfloat16)
            segp[:G_HALF] = xs[gs : gs + G_HALF]
            # [tiles, 64 graphs, 30, F] -> [tiles, F, 64*30]
            a = segp.reshape(N_TILES, TILE_G * K, F)
            out[c][:, half * F : (half + 1) * F, :] = a.transpose(0, 2, 1)
    return out


def _build_const(W1, b1, W2, b2, W3, b3, W4, b4, cw1, cb1, cw2, cb2, Wo, bo):
    import ml_dtypes

    cst = np.zeros((128, NCOL), np.float32)

    def bd(W):  # torch [out, in] -> block-diag lhsT [128, 2*out]
        o = W.shape[0]
        m = np.zeros((128, 2 * o), np.float32)
        m[0:64, 0:o] = W.T
        m[64:128, o : 2 * o] = W.T
        return m

    cst[:, OFF_W1 : OFF_W1 + 128] = bd(W1)
    cst[:, OFF_W2 : OFF_W2 + 128] = bd(W2)
    cst[:, OFF_W3 : OFF_W3 + 128] = bd(W3)
    w4m = np.zeros((128, 98), np.float32)
    w4m[0:64, 0:34] = W4.T
    w4m[64:128, 64:98] = W4.T
    cst[:, OFF_W4 : OFF_W4 + 98] = w4m

    def conv1_lhsT(ts):  # [34, 128] zero-padded; includes the 1/30 mean fold
        m = np.zeros((34, 128), np.float32)
        for ul, t in enumerate(ts):
            for oc in range(16):
                m[t : t + 5, ul * 16 + oc] = cw1[oc, 0, :] / float(K)
        return m

    # conv1 lhsT blocks live at partitions 0:34 (half A) AND 64:98 (half B)
    # so the head matmul lhsT base partition matches its pooled rhs.
    for r0 in (0, 64):
        cst[r0 : r0 + 34, OFF_CE1 : OFF_CE1 + 128] = conv1_lhsT(range(0, 16, 2))
        cst[r0 : r0 + 34, OFF_CO1 : OFF_CO1 + 128] = conv1_lhsT(range(1, 16, 2))
        cst[r0 : r0 + 34, OFF_CEM : OFF_CEM + 128] = conv1_lhsT(range(8, 24, 2))
        cst[r0 : r0 + 34, OFF_COM : OFF_COM + 128] = conv1_lhsT(range(9, 24, 2))
        cst[r0 : r0 + 34, OFF_CE2 : OFF_CE2 + 128] = conv1_lhsT(range(16, 30, 2))
        cst[r0 : r0 + 34, OFF_CO2 : OFF_CO2 + 128] = conv1_lhsT(range(17, 30, 2))

    def conv2_lhsT(tgs, us):  # [16*len(us), 128] zero-padded
        m = np.zeros((16 * len(us), 128), np.float32)
        for ri, u in enumerate(us):
            for ci, t in enumerate(tgs):
                kk = u - t
                if 0 <= kk < 5:
                    for ic in range(16):
                        m[ri * 16 + ic, ci * 32 : (ci + 1) * 32] = cw2[:, ic, kk]
        return m

    cst[0:128, OFF_C20 : OFF_C20 + 128] = conv2_lhsT(range(0, 4), range(0, 8))
    cst[0:128, OFF_C21 : OFF_C21 + 128] = conv2_lhsT(range(4, 8), range(4, 12))
    cst[0:112, OFF_C22 : OFF_C22 + 128] = conv2_lhsT(range(8, 11), range(8, 15))

    def wo_map(ts):  # [32*len(ts), 2]; undo the oc2-major flatten order
        m = np.zeros((32 * len(ts), 2), np.float32)
        for ci, t in enumerate(ts):
            for oc2 in range(32):
                m[ci * 32 + oc2, :] = Wo[:, oc2 * 11 + t]
        return m

    cst[64:98, OFF_ID : OFF_ID + 34] = np.eye(34, dtype=np.float32)
    cst[0:128, OFF_WO0 : OFF_WO0 + 2] = wo_map(range(0, 4))
    cst[0:128, OFF_WO1 : OFF_WO1 + 2] = wo_map(range(4, 8))
    cst[0:96, OFF_WO2 : OFF_WO2 + 2] = wo_map(range(8, 11))

    cstf = np.zeros((128, 8), np.float32)
    cstf[0:128, FB_B1] = np.concatenate([b1, b1])
    cstf[0:128, FB_B2] = np.concatenate([b2, b2])
    cstf[0:128, FB_B3] = np.concatenate([b3, b3])
    cstf[0:34, FB_B4] = b4
    cstf[64:98, FB_B4] = b4
    cstf[0:128, FB_C1B] = np.tile(cb1, 8)
    cstf[0:128, FB_C2B] = np.tile(cb2, 4)
    return cst.astype(ml_dtypes.bfloat16), cstf


def _numpy_forward(x, batch, W1, b1, W2, b2, W3, b3, W4, b4, cw1, cb1, cw2, cb2, Wo, bo):
    """General (slow) host fallback, used only if batch is not arange//100."""
    h = np.maximum(x @ W1.T + b1, 0)
    h = np.maximum(h @ W2.T + b2, 0)
    h = np.maximum(h @ W3.T + b3, 0)
    h = np.maximum(h @ W4.T + b4, 0)
    counts = np.bincount(batch, minlength=G).astype(np.float32)
    starts = np.cumsum(counts) - counts
    pos = np.arange(h.shape[0], dtype=np.float32) - starts[batch]
    mask = (pos < K).astype(np.float32)
    sums = np.zeros((G, h.shape[1]), np.float32)
    np.add.at(sums, batch, h * mask[:, None])
    denom = np.minimum(counts, float(K))
    pooled = sums / denom[:, None]
    c1 = np.zeros((G, 16, 30), np.float32)
    for t in range(30):
        c1[:, :, t] = pooled[:, t : t + 5] @ cw1[:, 0, :].T
    c1 = np.maximum(c1 + cb1[None, :, None], 0)
    m = np.maximum(c1[:, :, 0::2], c1[:, :, 1::2])  # [G, 16, 15]
    c2 = np.zeros((G, 32, 11), np.float32)
    for t in range(11):
        c2[:, :, t] = np.einsum("gik,oik->go", m[:, :, t : t + 5], cw2)
    c2 = np.maximum(c2 + cb2[None, :, None], 0)
    flat = c2.reshape(G, -1)
    return flat @ Wo.T + bo


def _run(inputs, trace=False, trace_kwargs=None):
    """Returns (y [10000, 2], BassKernelResults-or-None)."""
    x = np.ascontiguousarray(np.asarray(inputs["x"], dtype=np.float32))
    batch = np.asarray(inputs["batch"])
    names = ["W1", "b1", "W2", "b2", "W3", "b3", "W4", "b4",
             "cw1", "cb1", "cw2", "cb2", "Wo", "bo"]
    ws = [np.ascontiguousarray(np.asarray(inputs[n], dtype=np.float32)) for n in names]

    expected_batch = (np.arange(N, dtype=np.int64) // (N // G)).astype(batch.dtype)
    if batch.shape != (N,) or not np.array_equal(batch, expected_batch):
        return _numpy_forward(x, np.asarray(batch, np.int64), *ws), None

    from concourse.bass_utils import run_bass_kernel_spmd

    # the 'F' (fused L4 relu+halve) variant assumes b4 == 0
    variant = "F" if not np.any(ws[7]) else "noF"
    nc = _get_nc(variant)
    xt_all = _prep_x(x)
    cst, cstf = _build_const(*ws)
    in_maps = [{"xt": xt_all[c], "cst": cst, "cstf": cstf} for c in range(NCORE)]
    kw = {}
    if trace:
        kw["trace"] = True
        if trace_kwargs:
            kw["trace_kwargs"] = trace_kwargs
    res = run_bass_kernel_spmd(nc, in_maps, core_ids=list(range(NCORE)), **kw)

    bo = ws[-1]
    out = np.empty((G, 2), np.float32)
    # y layout: per head slot (c0, C): A chunk at cols [2*c0, 2*c0+C),
    # B chunk at [2*c0+C, 2*c0+2*C)
    for c in range(NCORE):
        yc = np.asarray(res.results[c]["y"], np.float32)
        base = c * G_CORE
        for (c0, C) in [(64 * i, 64) for i in range(N_TILES)]:
            na = min(C, max(0, G_HALF - c0))
            if na > 0:
                out[base + c0 : base + c0 + na] = yc[:, 2 * c0 : 2 * c0 + na].T
                out[base + G_HALF + c0 : base + G_HALF + c0 + na] = yc[
                    :, 2 * c0 + C : 2 * c0 + C + na].T
    return out + bo[None, :], res


def kernel(**inputs) -> np.ndarray:
    out, _ = _run(inputs)
    return out
